# revision 15
# baseline (speedup 1.0000x reference)
"""Trainium2 Bass kernel for exponential smoothing (EMA over time).

Math: out[b,t,h,d] = w_h^{t+1} v0[h,d] + sum_{j<=t} (1-w_h) w_h^{t-j} x[b,j,h,d]
(w = sigmoid(smoothing_weight)), i.e. the scan s_t = w s_{t-1} + (1-w) x_t with
s_{-1} = v0.

Kernel strategy (per core, data-parallel over batch: 16 batches / 8 cores,
2 per core). The whole problem is memory-bound, so the kernel moves bf16 in
both directions (tolerance is 2e-2; bf16 end-to-end lands ~3e-3):

  - The host pre-tiles the input into xp[128, 33, BPC, HD] bf16: partition p
    of chunk c holds time row c*127 + p - 1 (row 0 = carry slot). This makes
    every input DMA a [128 x multi-KB-contiguous] transfer (large descriptors)
    and removes all on-chip f32->bf16 casts. v0 (chunk 0) and the zero carries
    for the warm-up chunks are baked into the array on the host.
  - At bf16 the full input AND output fit in SBUF (66KB + 66KB of 208KB per
    partition): one resident in tile, one out tile, no group rotation.
  - Time chunks of C=127: each chunk is 8 per-head matmuls [128 x (2,64)];
    lhsT packs the carry-decay column w^(p+1) on top of lower-triangular
    (1-w)w^(p-j); columns permuted so the chunk's LAST output row lands at
    PSUM partition 0. The eviction of a chunk materializes that carry row
    in the out tile; gpsimd forwards it SBUF->SBUF into the next chunk's
    row-0 slot (gpsimd cannot read PSUM, but this keeps both PSUM-capable
    engines free for evictions).
  - The 33-chunk carry chain is split into 4 segments (starts 0/9/17/25).
    Segments 1..3 re-derive their carry with ONE zero-carry warm-up chunk
    (EMA influence decays ~w^127 -> far below bf16 noise) reading the SAME
    resident input tile -- no extra HBM traffic.
  - Output: PSUM evicts (f32->bf16) into the out tile, scalar engine for
    segs 0/1, vector for segs 2/3; 2-chunk strip DMAs store it via the
    scalar HWDGE queue as soon as both chunks are evicted (inputs ride the
    sync queue, so loads and stores overlap). Host un-permutes + upcasts.
"""

import numpy as np

B, T, H, D = 16, 4096, 8, 64
HD = H * D                    # 512
C = 127                       # chunk length (1 row reserved for the carry)
NFULL = T // C                # 32 full chunks
REM = T - NFULL * C           # 32-row tail chunk
NCH = NFULL + 1               # 33 chunk slots (incl. tail)
NCORES = 8
BPC = B // NCORES             # batches per core
SEG_BASES = [0, 9, 17, 25]
SEG_SIZES = [9, 8, 8, 8]      # seg 3 includes the tail chunk (c=32)
SEG_ENDS = {8, 16, 24}        # last full chunk of segs 0..2 (no carry out)
WARM_A = [8, 16, 24]          # zero-carry warm-up chunk for segs 1..3

_cache = {}


def _host_constants(smoothing_weight, v0, np_cdtype):
    """Parameter-derived constants, computed in fp64 on host."""
    w = 1.0 / (1.0 + np.exp(-np.asarray(smoothing_weight, np.float64)))  # [H,1]
    w = w[:, 0]

    def make_lhsT(n):
        # [H, n+1, n]; row 0 = w^(p+1) (carry decay), row 1+j = (1-w) w^(p-j)
        lt = np.zeros((H, n + 1, n), dtype=np.float64)
        p = np.arange(n)
        for hh in range(H):
            lt[hh, 0, :] = w[hh] ** (p + 1)
            for j in range(n):
                lt[hh, 1 + j, j:] = (1.0 - w[hh]) * w[hh] ** (p[j:] - j)
        return lt.astype(np_cdtype)

    wt = make_lhsT(C)          # [H, 128, 127]
    # permute out rows: [last, 0..last-1] so the carry row lands at PSUM
    # partition 0 (aligned); the host un-permutes
    wt = np.concatenate([wt[:, :, C - 1:], wt[:, :, :C - 1]], axis=2)
    wt2 = make_lhsT(REM)       # [H, 33, 32] (tail: no carry out, unpermuted)
    # pad M to 128 (zero column): Fast Weight Load needs NumWeights == 128;
    # the extra PSUM row evicts as zeros the host discards
    wt = np.concatenate([wt, np.zeros((H, C + 1, 1), wt.dtype)], axis=2)
    # [K, H, M] layout so the on-chip weight DMA is contiguous per partition
    wt = np.ascontiguousarray(wt.transpose(1, 0, 2))    # [128, 8, 128]
    wt2 = np.ascontiguousarray(wt2.transpose(1, 0, 2))  # [33, 8, 32]
    v0row = np.asarray(v0, np.float32).reshape(HD).astype(np_cdtype)
    return wt, wt2, v0row


def _host_inputs(values, smoothing_weight, v0):
    import ml_dtypes
    bf16 = ml_dtypes.bfloat16
    wt, wt2, v0row = _host_constants(smoothing_weight, v0, bf16)
    x = np.asarray(values, np.float32).reshape(B, T, HD).astype(bf16)
    # xp[p, c, b, :] = xpad[b, c*C + p, :], xpad = [junk-row-0, x, zero pad]
    idx = (np.arange(NCH) * C)[:, None] + np.arange(C + 1)[None, :]  # [33,128]
    in_maps = []
    for core in range(NCORES):
        xpad = np.zeros((BPC, 1 + T + C, HD), bf16)
        xpad[:, 1:T + 1] = x[core * BPC:(core + 1) * BPC]
        xp = xpad[:, idx, :]                                 # [2, 33, 128, 512]
        xp = np.ascontiguousarray(xp.transpose(2, 1, 0, 3))  # [128, 33, 2, 512]
        xp[0, 0, :, :] = v0row           # chunk 0 carry = v0
        for wa in WARM_A:
            xp[0, wa, :, :] = 0          # zero carry for warm-up chunks
        in_maps.append({"xp": xp, "wt": wt, "wt2": wt2})
    return in_maps


def _host_output(ops):
    outs = []
    for op in ops:                       # [128, 33, 2, 512] bf16
        # PSUM partition 0 = chunk's last time row, 1..126 = rows 0..125
        full = np.roll(op[0:C, 0:NFULL], -1, axis=0)         # [127, 32, 2, 512]
        fullt = full.transpose(2, 1, 0, 3).reshape(BPC, NFULL * C, HD)
        tail = op[0:REM, NFULL].transpose(1, 0, 2)           # [2, 32, 512]
        outs.append(np.concatenate([fullt, tail], axis=1))
    out = np.concatenate(outs, axis=0).astype(np.float32)
    return out.reshape(B, T, H, D)


def _build_program():
    import concourse.tile as tile
    from concourse import bacc, mybir
    from contextlib import ExitStack

    cdtype = mybir.dt.bfloat16
    f32 = mybir.dt.float32

    nc = bacc.Bacc("TRN2", target_bir_lowering=False, debug=False,
                   num_devices=NCORES)

    xp_d = nc.dram_tensor("xp", [C + 1, NCH, BPC, HD], cdtype,
                          kind="ExternalInput").ap()
    wt_d = nc.dram_tensor("wt", [C + 1, H, C + 1], cdtype,
                          kind="ExternalInput").ap()
    wt2_d = nc.dram_tensor("wt2", [REM + 1, H, REM], cdtype,
                           kind="ExternalInput").ap()
    out_d = nc.dram_tensor("out", [C + 1, NCH, BPC, HD], cdtype,
                           kind="ExternalOutput").ap()

    with tile.TileContext(nc) as tc, ExitStack() as ctx:
        consts = ctx.enter_context(tc.tile_pool(name="consts", bufs=1))
        io_pool = ctx.enter_context(tc.tile_pool(name="io", bufs=1))
        psum_pool = ctx.enter_context(tc.tile_pool(name="psum", bufs=4,
                                                   space="PSUM"))

        wt_s = consts.tile([C + 1, H, C + 1], cdtype)   # [128, 8, 128]
        wt2_s = consts.tile([REM + 1, H, REM], cdtype)  # [33, 8, 32]
        in_s = io_pool.tile([C + 1, NCH, BPC, HD], cdtype, tag="in",
                            name="in_s")
        out_s = io_pool.tile([C + 1, NCH, BPC, HD], cdtype, tag="out",
                             name="out_s")

        def load_strip(c0, c1):
            nc.sync.dma_start(out=in_s[:, c0:c1, :, :], in_=xp_d[:, c0:c1])

        # Input strips on the sync HWDGE queue, ordered so chunk 0 and the
        # warm-up feed chunks land first, then round-robin across segments.
        nc.sync.dma_start(out=wt_s[:], in_=wt_d)
        load_strip(0, 1)
        load_strip(8, 10)
        nc.sync.dma_start(out=wt2_s[:], in_=wt2_d)
        load_strip(16, 18)
        load_strip(24, 26)
        for (a, b) in ((1, 3), (10, 12), (18, 20), (26, 28),
                       (3, 5), (12, 14), (20, 22), (28, 30),
                       (5, 8), (14, 16), (22, 24), (30, 33)):
            load_strip(a, b)

        # tail chunk only fills partitions 0..31 of the out tile; zero the
        # rest once so the bulk store reads defined data (gpsimd instructions
        # are capped at 32 partitions each)
        for p0 in range(REM, C + 1, 32):
            nc.gpsimd.memset(out_s[p0:p0 + 32, NFULL, :, :], 0.0)

        def bhd(ap):
            # view a [p, b, (h d)] slice as [p, h, b, d] (PSUM layout order)
            return ap.rearrange("p b (h d) -> p h b d", h=H)

        # split the PSUM-drain work symmetrically: segs 0/1 evict on scalar
        # and carry on vector, segs 2/3 the other way around, so each engine
        # does 2 evictions + 2 carry copies per round (~4.6us) and a chain's
        # carry never waits behind its own eviction on the same queue
        evict_eng = {0: nc.scalar.copy, 1: nc.scalar.copy,
                     2: nc.vector.tensor_copy, 3: nc.vector.tensor_copy}
        carry_eng = {0: nc.vector.tensor_copy, 1: nc.vector.tensor_copy,
                     2: nc.scalar.copy, 3: nc.scalar.copy}

        def chunk_step(c, seg, carry=True, warm=False):
            ps = psum_pool.tile([C + 1, H, BPC, D], f32, tag="ps")
            for hh in range(H):
                nc.tensor.matmul(
                    out=ps[:, hh, :, :],
                    lhsT=wt_s[:, hh, :],
                    rhs=in_s[:, c, :, hh * D:(hh + 1) * D],
                    start=True, stop=True,
                )
            if carry:
                # carry straight from PSUM row 0 into the next chunk's row-0
                # slot; this is the chain-critical op (~1.2us)
                carry_eng[seg](bhd(in_s[0:1, c + 1, :, :]), ps[0:1, :, :, :])
            if not warm:
                evict_eng[seg](bhd(out_s[:, c, :, :]), ps[:, :, :, :])

        def tail_step():
            ps = psum_pool.tile([C + 1, H, BPC, D], f32, tag="ps")
            for hh in range(H):
                nc.tensor.matmul(
                    out=ps[0:REM, hh, :, :],
                    lhsT=wt2_s[:, hh, :],
                    rhs=in_s[0:REM + 1, NFULL, :, hh * D:(hh + 1) * D],
                    start=True, stop=True,
                )
            nc.vector.tensor_copy(bhd(out_s[0:REM, NFULL, :, :]),
                                  ps[0:REM, :, :, :])

        # chunk 0 first (fast start), then the three 1-chunk warm-ups (row 0
        # of chunks 8/16/24 is host-zeroed; EMA influence decays as w^127)
        chunk_step(0, 0)
        for s in (1, 2, 3):
            chunk_step(SEG_BASES[s] - 1, s, warm=True)

        # main rounds, round-robin across the 4 segments; 2-chunk output
        # strips are stored (scalar HWDGE queue) as soon as both are evicted
        for r in range(9):
            # segment order 0,2,1,3 alternates carry/evict on each engine
            # queue, so a chain-critical carry waits behind at most one
            # eviction
            for s in (0, 2, 1, 3):
                if r >= SEG_SIZES[s] or (r == 0 and s == 0):
                    continue
                c = SEG_BASES[s] + r
                if c == NFULL:
                    tail_step()
                else:
                    chunk_step(c, s, carry=c not in SEG_ENDS)
            # store early chunks in 2-chunk strips at odd rounds, the last
            # two chunks of each segment singly (r6/r7) so the post-compute
            # store tail stays small
            if r in (1, 3, 5):
                for s in range(4):
                    a = SEG_BASES[s] + r - 1
                    nc.sync.dma_start(out=out_d[:, a:a + 2],
                                      in_=out_s[:, a:a + 2, :, :])
            elif r in (6, 7):
                for s in range(4):
                    a = SEG_BASES[s] + r
                    nc.sync.dma_start(out=out_d[:, a:a + 1],
                                      in_=out_s[:, a:a + 1, :, :])
            elif r == 8:
                nc.sync.dma_start(out=out_d[:, 8:9],
                                  in_=out_s[:, 8:9, :, :])

    nc.compile()
    return nc


def _get_program():
    if "p" not in _cache:
        _cache["p"] = _build_program()
    return _cache["p"]


def kernel(values, smoothing_weight, v0):
    from concourse.bass_utils import run_bass_kernel_spmd

    nc = _get_program()
    in_maps = _host_inputs(values, smoothing_weight, v0)
    res = run_bass_kernel_spmd(nc, in_maps, list(range(NCORES)))
    return _host_output([res.results[i]["out"] for i in range(NCORES)])
